# revision 1
# baseline (speedup 1.0000x reference)
"""MQA attention kernel v3 for Trainium2 (8 NeuronCores, Bass/Tile).

Problem: Q [2,16,2048,64], K/V [2,1,2048,64] fp32, out = softmax(QK^T/8) V.
Sharding: 32 (batch, head) pairs over 8 cores -> 4 heads/core; one batch's
K/V per core.

v3 = v2's data path with a software-pipelined emission order:
  - bf16 everywhere; Q^T/K^T via gpsimd DMA-cast + xbar DMA transposes of
    [128,128] chunk-pairs (zero PE/DVE cost). kTs + half-swapped kTs2 give
    every chunk in both partition halves.
  - Unit = (head, 512-col q-block): 16 score steps per unit; each step is
    one [64-deep] QK matmul pair (parities in the two PE row halves), one
    exp instruction (ScalarE table-exp for 12 steps, VectorE Schraudolph
    int16->bf16 for 4), plus TWO PV-chain matmuls of the PREVIOUS unit
    threaded between - so ScalarE never idles at unit boundaries.
  - PV: full-128-deep accumulation chains (one PSUM bank per col-group,
    LDWEIGHTS hides in the background weight buffer).
  - Output: PE transpose -> DVE PSUM->SBUF copy -> gpsimd normalize_recip
    (the only engine with slack) -> DMA store.

The q columns are processed in a (parity, block) interleaved order; the
output store APs undo the permutation (col-group (b, parity p, sub t)
holds global q with q mod 16 == 8b + 2t + p).
"""

import numpy as np

import concourse.bass as bass
import concourse.mybir as mybir
import concourse.tile as tile
from concourse import bacc
from concourse.bass_utils import run_bass_kernel_spmd
from concourse.masks import make_identity

B, H, S, D = 2, 16, 2048, 64
N_CORES = 8
HPC = (B * H) // N_CORES
P = 128
NJ = S // P
NG = NJ // 2
QB = 512
LAG = 4
SCALE = 1.0 / float(D) ** 0.5
F32 = mybir.dt.float32
BF16 = mybir.dt.bfloat16
I16 = mybir.dt.int16

LOG2E = 1.4426950408889634
A16 = float((1 << 7) * LOG2E * SCALE)
B16 = float(127.0 * (1 << 7) - 7.42)

# exp steps (tt, jj) run on the DVE via Schraudolph; listing both tt of a
# jj approximates chunks {2jj, 2jj+1} fully.
DVE_TILES = ((0, 1), (0, 3), (0, 5), (1, 1), (1, 3))

_CACHED = {}
DEFAULT_CFG = {}


def _build_module(**cfg):
    nc = bacc.Bacc(None)
    q = nc.dram_tensor("q", [HPC, S, D], F32, kind="ExternalInput")
    k = nc.dram_tensor("k", [S, D], F32, kind="ExternalInput")
    v = nc.dram_tensor("v", [S, D], F32, kind="ExternalInput")
    o = nc.dram_tensor("o", [HPC, S, D], F32, kind="ExternalOutput")
    _trace_body(nc, q, k, v, o, **cfg)
    nc.compile()
    return nc


def _trace_body(nc, q, k, v, o, sg_bufs=3, pv_bufs=2, pt_bufs=3, dve_tiles=None):
    dve_tiles = DVE_TILES if dve_tiles is None else dve_tiles
    with tile.TileContext(nc) as tc:
        with (
            tc.tile_pool(name="const", bufs=1) as cpool,
            tc.tile_pool(name="qb", bufs=2) as qpool,
            tc.tile_pool(name="ptb", bufs=pt_bufs) as ptpool,
            tc.tile_pool(name="wk", bufs=2) as wpool,
            tc.tile_pool(name="sg", bufs=sg_bufs, space="PSUM") as sgpool,
            tc.tile_pool(name="pv", bufs=pv_bufs, space="PSUM") as pvpool,
        ):
            k_bf = cpool.tile([P, NJ, D], BF16)
            k_src = k.rearrange("(p c) d -> p c d", p=P)
            nc.gpsimd.dma_start(k_bf[:, 0 : NJ // 2, :], k_src[:, 0 : NJ // 2, :])
            nc.gpsimd.dma_start(k_bf[:, NJ // 2 :, :], k_src[:, NJ // 2 :, :])
            kTs = cpool.tile([P, NG, P], BF16)
            kTs2 = cpool.tile([P, NG, P], BF16)
            for g in range(NG):
                nc.scalar.dma_start_transpose(
                    kTs[:, g, :], k_bf[:, 2 * g : 2 * g + 2, :]
                )
            nc.sync.dma_start(kTs2[0:64, :, :], kTs[64:P, :, :])
            nc.sync.dma_start(kTs2[64:P, :, :], kTs[0:64, :, :])

            identity = cpool.tile([P, P], F32)
            make_identity(nc, identity)
            identity_bf = cpool.tile([P, P], BF16)
            nc.vector.tensor_copy(identity_bf[:], identity[:])

            v_nat = cpool.tile([P, NJ, D], F32)
            nc.sync.dma_start(v_nat[:], v.rearrange("(p c) d -> p c d", p=P))
            vp = cpool.tile([P, NJ, D + 1], BF16)
            nc.gpsimd.memset(vp[:, :, D], 1.0)
            nc.vector.tensor_copy(vp[:, :, 0:D], v_nat[:])

            def load_qT(h):
                q_bf = qpool.tile([P, NJ, D], BF16, tag="qbf", name=f"qbf{h}")
                q_src = q[h].rearrange("(p c) d -> p c d", p=P)
                nc.gpsimd.dma_start(q_bf[:, 0 : NJ // 2, :], q_src[:, 0 : NJ // 2, :])
                nc.gpsimd.dma_start(q_bf[:, NJ // 2 :, :], q_src[:, NJ // 2 :, :])
                qTs = qpool.tile([P, NG, P], BF16, tag="qTs", name=f"qTs{h}")
                for g in range(NG):
                    nc.sync.dma_start_transpose(
                        qTs[:, g, :], q_bf[:, 2 * g : 2 * g + 2, :]
                    )
                return qTs

            def pt_idx(p, c):
                jj, jp = divmod(c, 2)
                if p == 0:
                    return (0, jj, 0) if jp == 0 else (1, jj, 0)
                return (0, jj, 1) if jp == 1 else (1, jj, 1)

            qTs_cur = load_qT(0)

            # ---- steady-state pipeline over units u = (h, b); PV of unit
            # u-1 threads through unit u's score steps ----
            units = [(h, b) for h in range(HPC) for b in range(2)]
            qTs_of = {0: qTs_cur}
            pT_of = {}
            pv_of = {}

            def pt_idx(p, c):
                jj, jp = divmod(c, 2)
                if p == 0:
                    return (0, jj, 0) if jp == 0 else (1, jj, 0)
                return (0, jj, 1) if jp == 1 else (1, jj, 1)

            def emit_output(h, b, p, pv):
                otr = pvpool.tile(
                    [P, 4, D + 1], BF16, tag="pv", name=f"otr{h}_{b}_{p}",
                    padded_shape=[P, 4, QB // 2],
                )
                oev = wpool.tile([D + 1, QB], BF16, tag="oev", name=f"oev{h}_{b}_{p}")
                nc.vector.tensor_copy(oev[:], pv[:])
                for t in range(4):
                    nc.tensor.transpose(
                        otr[:, t, :],
                        oev[:, P * t : P * (t + 1)],
                        identity_bf[0 : D + 1, 0 : D + 1],
                    )
                osb = wpool.tile([P, 4, D + 1], F32, tag="osb", name=f"osb{h}_{b}_{p}")
                nc.vector.tensor_copy(osb[:], otr[:])
                oout = wpool.tile([P, 4, D], F32, tag="oout", name=f"oout{h}_{b}_{p}")
                for t in range(4):
                    nc.gpsimd.normalize_recip(
                        oout[:, t, :], osb[:, t, 0:D], osb[:, t, D : D + 1]
                    )
                nc.sync.dma_start(
                    o[h].rearrange("(p c) d -> p c d", p=P)[
                        :, 8 * b + p : 8 * b + 8 : 2, :
                    ],
                    oout[:],
                )

            def emit_pv_steps(u_prev, s):
                h, b = units[u_prev]
                p = s // 8
                pT = pT_of[u_prev]
                if s % 8 == 0:
                    pv_of[u_prev * 2 + p] = pvpool.tile(
                        [D + 1, QB], F32, tag="pv", name=f"pv{h}_{b}_{p}",
                        padded_shape=[P, QB],
                    )
                pv = pv_of[u_prev * 2 + p]
                for c in (2 * (s % 8), 2 * (s % 8) + 1):
                    tt, jj, sl = pt_idx(p, c)
                    nc.tensor.matmul(
                        pv[:],
                        lhsT=vp[:, c, :],
                        rhs=pT[:, tt, jj, sl, :],
                        start=(c == 0),
                        stop=(c == NJ - 1),
                        skip_group_check=True,
                    )
                if s % 8 == 7:
                    emit_output(h, b, p, pv)

            for u, (h, b) in enumerate(units):
                if b == 0:
                    qTs = qTs_of[h]
                pT = ptpool.tile([P, 2, NG, 2, QB], BF16, tag="pT", name=f"pT{h}_{b}")
                pT_of[u] = pT
                for s in range(16):
                    jj, tt = s % 8, s // 8
                    kt_src = kTs if tt == 0 else kTs2
                    sg = sgpool.tile([P, 2, QB], F32, tag="sg", name=f"sg{h}_{b}_{s}")
                    nc.tensor.matmul(
                        sg[:, 0, :],
                        lhsT=kt_src[0:64, jj, :],
                        rhs=qTs[0:64, 4 * b : 4 * (b + 1), :],
                        start=True, stop=True,
                    )
                    nc.tensor.matmul(
                        sg[:, 1, :],
                        lhsT=kt_src[64:P, jj, :],
                        rhs=qTs[64:P, 4 * b : 4 * (b + 1), :],
                        start=True, stop=True,
                    )
                    if u > 0:
                        emit_pv_steps(u - 1, s)
                    out_ap = pT[:, tt, jj, :, :]
                    if (tt, jj) in dve_tiles:
                        nc.vector.tensor_scalar(
                            out_ap.bitcast(I16),
                            sg[:],
                            A16,
                            B16,
                            mybir.AluOpType.mult,
                            mybir.AluOpType.add,
                        )
                    else:
                        nc.scalar.activation(
                            out_ap,
                            sg[:],
                            mybir.ActivationFunctionType.Exp,
                            scale=SCALE,
                        )
                    if s == 3 and b == 1 and h + 1 < HPC:
                        qTs_of[h + 1] = load_qT(h + 1)

            for s in range(16):
                emit_pv_steps(len(units) - 1, s)


def _get_module(**cfg):
    key = tuple(sorted(cfg.items()))
    if key not in _CACHED:
        _CACHED[key] = _build_module(**cfg)
    return _CACHED[key]


def make_in_maps(Q, K, V):
    Q = np.asarray(Q, dtype=np.float32)
    K = np.asarray(K, dtype=np.float32)
    V = np.asarray(V, dtype=np.float32)
    in_maps = []
    for c in range(N_CORES):
        b = c // (N_CORES // B)
        h0 = HPC * (c % (N_CORES // B))
        in_maps.append(
            {
                "q": np.ascontiguousarray(Q[b, h0 : h0 + HPC]),
                "k": np.ascontiguousarray(K[b, 0]),
                "v": np.ascontiguousarray(V[b, 0]),
            }
        )
    return in_maps


def assemble_output(results):
    out = np.empty((B, H, S, D), dtype=np.float32)
    for c in range(N_CORES):
        b = c // (N_CORES // B)
        h0 = HPC * (c % (N_CORES // B))
        out[b, h0 : h0 + HPC] = results[c]["o"]
    return out


def kernel(Q, K, V):
    nc = _get_module(**DEFAULT_CFG)
    res = run_bass_kernel_spmd(nc, make_in_maps(Q, K, V), core_ids=list(range(N_CORES)))
    return assemble_output(res.results)



# revision 5
# speedup vs baseline: 1.0767x; 1.0767x over previous
"""MQA attention kernel v3 for Trainium2 (8 NeuronCores, Bass/Tile).

Problem: Q [2,16,2048,64], K/V [2,1,2048,64] fp32, out = softmax(QK^T/8) V.
Sharding: 32 (batch, head) pairs over 8 cores -> 4 heads/core; one batch's
K/V per core.

v3 = v2's data path with a software-pipelined emission order:
  - bf16 everywhere; Q^T/K^T via gpsimd DMA-cast + xbar DMA transposes of
    [128,128] chunk-pairs (zero PE/DVE cost). kTs + half-swapped kTs2 give
    every chunk in both partition halves.
  - Unit = (head, 512-col q-block): 16 score steps per unit; each step is
    one [64-deep] QK matmul pair (parities in the two PE row halves), one
    exp instruction (ScalarE table-exp for 12 steps, VectorE Schraudolph
    int16->bf16 for 4), plus TWO PV-chain matmuls of the PREVIOUS unit
    threaded between - so ScalarE never idles at unit boundaries.
  - PV: full-128-deep accumulation chains (one PSUM bank per col-group,
    LDWEIGHTS hides in the background weight buffer).
  - Output: PE transpose -> DVE PSUM->SBUF copy -> gpsimd normalize_recip
    (the only engine with slack) -> DMA store.

The q columns are processed in a (parity, block) interleaved order; the
output store APs undo the permutation (col-group (b, parity p, sub t)
holds global q with q mod 16 == 8b + 2t + p).
"""

import numpy as np

import concourse.bass as bass
import concourse.mybir as mybir
import concourse.tile as tile
from concourse import bacc
from concourse.bass_utils import run_bass_kernel_spmd
from concourse.masks import make_identity

B, H, S, D = 2, 16, 2048, 64
N_CORES = 8
HPC = (B * H) // N_CORES
P = 128
NJ = S // P
NG = NJ // 2
QB = 512
LAG = 4
SCALE = 1.0 / float(D) ** 0.5
F32 = mybir.dt.float32
BF16 = mybir.dt.bfloat16
I16 = mybir.dt.int16

LOG2E = 1.4426950408889634
A16 = float((1 << 7) * LOG2E * SCALE)
B16 = float(127.0 * (1 << 7) - 7.42)

# exp steps (tt, jj) run on the DVE via Schraudolph; listing both tt of a
# jj approximates chunks {2jj, 2jj+1} fully.
DVE_TILES = ((0, 1), (0, 3), (0, 5), (1, 1), (1, 3), (1, 5))

_CACHED = {}
DEFAULT_CFG = {}


def _build_module(**cfg):
    nc = bacc.Bacc(None)
    q = nc.dram_tensor("q", [HPC, S, D], F32, kind="ExternalInput")
    k = nc.dram_tensor("k", [S, D], F32, kind="ExternalInput")
    v = nc.dram_tensor("v", [S, D], F32, kind="ExternalInput")
    o = nc.dram_tensor("o", [HPC, S, D], F32, kind="ExternalOutput")
    _trace_body(nc, q, k, v, o, **cfg)
    nc.compile()
    return nc


def _trace_body(nc, q, k, v, o, sg_bufs=3, pv_bufs=2, pt_bufs=3, dve_tiles=None):
    dve_tiles = DVE_TILES if dve_tiles is None else dve_tiles
    with tile.TileContext(nc) as tc:
        with (
            tc.tile_pool(name="const", bufs=1) as cpool,
            tc.tile_pool(name="qb", bufs=2) as qpool,
            tc.tile_pool(name="ptb", bufs=pt_bufs) as ptpool,
            tc.tile_pool(name="wk", bufs=2) as wpool,
            tc.tile_pool(name="sg", bufs=sg_bufs, space="PSUM") as sgpool,
            tc.tile_pool(name="pv", bufs=pv_bufs, space="PSUM") as pvpool,
        ):
            # ---- prologue v4: HWDGE fp32 loads + engine casts; kT/qT(h0)
            # transposed on the (otherwise idle) PE through PSUM slots the sg
            # pool will reuse. This keeps the Scalar queue free of 1.2us DMA
            # transposes so exp can start with the first score matmul. ----
            identity = cpool.tile([P, P], F32)
            make_identity(nc, identity)
            identity_bf = cpool.tile([P, P], BF16)
            nc.vector.tensor_copy(identity_bf[:], identity[:])

            k_src = k.rearrange("(p c) d -> p c d", p=P)
            k_nat = cpool.tile([P, NJ, D], F32)
            nc.sync.dma_start(k_nat[:, 0 : NJ // 2, :], k_src[:, 0 : NJ // 2, :])
            nc.sync.dma_start(k_nat[:, NJ // 2 :, :], k_src[:, NJ // 2 :, :])
            k_bf = cpool.tile([P, NJ, D], BF16)
            nc.scalar.activation(
                k_bf[:, 0 : NJ // 2, :], k_nat[:, 0 : NJ // 2, :],
                mybir.ActivationFunctionType.Copy,
            )
            nc.vector.tensor_copy(k_bf[:, NJ // 2 :, :], k_nat[:, NJ // 2 :, :])

            q0_src = q[0].rearrange("(p c) d -> p c d", p=P)
            q0_nat = cpool.tile([P, NJ, D], F32)
            nc.sync.dma_start(q0_nat[:, 0 : NJ // 2, :], q0_src[:, 0 : NJ // 2, :])
            nc.sync.dma_start(q0_nat[:, NJ // 2 :, :], q0_src[:, NJ // 2 :, :])

            v_nat = cpool.tile([P, NJ, D], F32)
            nc.sync.dma_start(v_nat[:], v.rearrange("(p c) d -> p c d", p=P))

            def transpose_to(dst_sb, src_bf, psname):
                ps = sgpool.tile(
                    [P, NG, P], BF16, tag="sg", name=psname,
                    padded_shape=[P, 2 * NG, P],
                )
                for g in range(NG):
                    nc.tensor.transpose(
                        ps[:, g, :], src_bf[:, 2 * g : 2 * g + 2, :], identity_bf[:]
                    )
                nc.scalar.activation(
                    dst_sb[:, 0 : NG // 2, :], ps[:, 0 : NG // 2, :],
                    mybir.ActivationFunctionType.Copy,
                )
                nc.vector.tensor_copy(dst_sb[:, NG // 2 :, :], ps[:, NG // 2 :, :])

            kTs = cpool.tile([P, NG, P], BF16)
            transpose_to(kTs, k_bf, "kt_ps")
            kTs2 = cpool.tile([P, NG, P], BF16)
            nc.sync.dma_start(kTs2[0:64, :, :], kTs[64:P, :, :])
            nc.sync.dma_start(kTs2[64:P, :, :], kTs[0:64, :, :])

            vp = cpool.tile([P, NJ, D + 1], BF16)
            nc.gpsimd.memset(vp[:, :, D], 1.0)
            nc.vector.tensor_copy(vp[:, :, 0:D], v_nat[:])

            def load_qT(h):
                q_bf = qpool.tile([P, NJ, D], BF16, tag="qbf", name=f"qbf{h}")
                q_src = q[h].rearrange("(p c) d -> p c d", p=P)
                if h == 0:
                    nc.vector.tensor_copy(q_bf[:, 0 : NJ // 2, :], q0_nat[:, 0 : NJ // 2, :])
                    nc.scalar.activation(
                        q_bf[:, NJ // 2 :, :], q0_nat[:, NJ // 2 :, :],
                        mybir.ActivationFunctionType.Copy,
                    )
                else:
                    nc.gpsimd.dma_start(q_bf[:, 0 : NJ // 2, :], q_src[:, 0 : NJ // 2, :])
                    nc.gpsimd.dma_start(q_bf[:, NJ // 2 :, :], q_src[:, NJ // 2 :, :])
                qTs = qpool.tile([P, NG, P], BF16, tag="qTs", name=f"qTs{h}")
                if h == 0:
                    transpose_to(qTs, q_bf, "qt_ps0")
                else:
                    for g in range(NG):
                        nc.sync.dma_start_transpose(
                            qTs[:, g, :], q_bf[:, 2 * g : 2 * g + 2, :]
                        )
                return qTs

            def pt_idx(p, c):
                jj, jp = divmod(c, 2)
                if p == 0:
                    return (0, jj, 0) if jp == 0 else (1, jj, 0)
                return (0, jj, 1) if jp == 1 else (1, jj, 1)

            qTs_cur = load_qT(0)

            # ---- steady-state pipeline over units u = (h, b); PV of unit
            # u-1 threads through unit u's score steps ----
            units = [(h, b) for h in range(HPC) for b in range(2)]
            qTs_of = {0: qTs_cur}
            pT_of = {}
            pv_of = {}

            def pt_idx(p, c):
                jj, jp = divmod(c, 2)
                if p == 0:
                    return (0, jj, 0) if jp == 0 else (1, jj, 0)
                return (0, jj, 1) if jp == 1 else (1, jj, 1)

            def emit_output(h, b, p, pv):
                otr = pvpool.tile(
                    [P, 4, D + 1], BF16, tag="pv", name=f"otr{h}_{b}_{p}",
                    padded_shape=[P, 4, QB // 2],
                )
                oev = wpool.tile([D + 1, QB], BF16, tag="oev", name=f"oev{h}_{b}_{p}")
                nc.vector.tensor_copy(oev[:], pv[:])
                for t in range(4):
                    nc.tensor.transpose(
                        otr[:, t, :],
                        oev[:, P * t : P * (t + 1)],
                        identity_bf[0 : D + 1, 0 : D + 1],
                    )
                osb = wpool.tile([P, 4, D + 1], F32, tag="osb", name=f"osb{h}_{b}_{p}")
                nc.vector.tensor_copy(osb[:], otr[:])
                oout = wpool.tile([P, 4, D], F32, tag="oout", name=f"oout{h}_{b}_{p}")
                for t in range(4):
                    nc.gpsimd.normalize_recip(
                        oout[:, t, :], osb[:, t, 0:D], osb[:, t, D : D + 1]
                    )
                nc.sync.dma_start(
                    o[h].rearrange("(p c) d -> p c d", p=P)[
                        :, 8 * b + p : 8 * b + 8 : 2, :
                    ],
                    oout[:],
                )

            def emit_pv_steps(u_prev, s):
                h, b = units[u_prev]
                p = s // 8
                pT = pT_of[u_prev]
                if s % 8 == 0:
                    pv_of[u_prev * 2 + p] = pvpool.tile(
                        [D + 1, QB], F32, tag="pv", name=f"pv{h}_{b}_{p}",
                        padded_shape=[P, QB],
                    )
                pv = pv_of[u_prev * 2 + p]
                for c in (2 * (s % 8), 2 * (s % 8) + 1):
                    tt, jj, sl = pt_idx(p, c)
                    nc.tensor.matmul(
                        pv[:],
                        lhsT=vp[:, c, :],
                        rhs=pT[:, tt, jj, sl, :],
                        start=(c == 0),
                        stop=(c == NJ - 1),
                        skip_group_check=True,
                    )
                if s % 8 == 7:
                    emit_output(h, b, p, pv)

            for u, (h, b) in enumerate(units):
                if b == 0:
                    qTs = qTs_of[h]
                pT = ptpool.tile([P, 2, NG, 2, QB], BF16, tag="pT", name=f"pT{h}_{b}")
                pT_of[u] = pT
                for s in range(16):
                    jj, tt = s % 8, s // 8
                    kt_src = kTs if tt == 0 else kTs2
                    sg = sgpool.tile([P, 2, QB], F32, tag="sg", name=f"sg{h}_{b}_{s}")
                    nc.tensor.matmul(
                        sg[:, 0, :],
                        lhsT=kt_src[0:64, jj, :],
                        rhs=qTs[0:64, 4 * b : 4 * (b + 1), :],
                        start=True, stop=True,
                    )
                    nc.tensor.matmul(
                        sg[:, 1, :],
                        lhsT=kt_src[64:P, jj, :],
                        rhs=qTs[64:P, 4 * b : 4 * (b + 1), :],
                        start=True, stop=True,
                    )
                    if u > 0:
                        emit_pv_steps(u - 1, s)
                    out_ap = pT[:, tt, jj, :, :]
                    if (tt, jj) in dve_tiles:
                        nc.vector.tensor_scalar(
                            out_ap.bitcast(I16),
                            sg[:],
                            A16,
                            B16,
                            mybir.AluOpType.mult,
                            mybir.AluOpType.add,
                        )
                    else:
                        nc.scalar.activation(
                            out_ap,
                            sg[:],
                            mybir.ActivationFunctionType.Exp,
                            scale=SCALE,
                        )
                    if s == 3 and b == 1 and h + 1 < HPC:
                        qTs_of[h + 1] = load_qT(h + 1)

            for s in range(16):
                emit_pv_steps(len(units) - 1, s)


def _get_module(**cfg):
    key = tuple(sorted(cfg.items()))
    if key not in _CACHED:
        _CACHED[key] = _build_module(**cfg)
    return _CACHED[key]


def make_in_maps(Q, K, V):
    Q = np.asarray(Q, dtype=np.float32)
    K = np.asarray(K, dtype=np.float32)
    V = np.asarray(V, dtype=np.float32)
    in_maps = []
    for c in range(N_CORES):
        b = c // (N_CORES // B)
        h0 = HPC * (c % (N_CORES // B))
        in_maps.append(
            {
                "q": np.ascontiguousarray(Q[b, h0 : h0 + HPC]),
                "k": np.ascontiguousarray(K[b, 0]),
                "v": np.ascontiguousarray(V[b, 0]),
            }
        )
    return in_maps


def assemble_output(results):
    out = np.empty((B, H, S, D), dtype=np.float32)
    for c in range(N_CORES):
        b = c // (N_CORES // B)
        h0 = HPC * (c % (N_CORES // B))
        out[b, h0 : h0 + HPC] = results[c]["o"]
    return out


def kernel(Q, K, V):
    nc = _get_module(**DEFAULT_CFG)
    res = run_bass_kernel_spmd(nc, make_in_maps(Q, K, V), core_ids=list(range(N_CORES)))
    return assemble_output(res.results)



# revision 10
# speedup vs baseline: 1.0800x; 1.0031x over previous
"""MQA attention kernel v3 for Trainium2 (8 NeuronCores, Bass/Tile).

Problem: Q [2,16,2048,64], K/V [2,1,2048,64] fp32, out = softmax(QK^T/8) V.
Sharding: 32 (batch, head) pairs over 8 cores -> 4 heads/core; one batch's
K/V per core.

v3 = v2's data path with a software-pipelined emission order:
  - bf16 everywhere; Q^T/K^T via gpsimd DMA-cast + xbar DMA transposes of
    [128,128] chunk-pairs (zero PE/DVE cost). kTs + half-swapped kTs2 give
    every chunk in both partition halves.
  - Unit = (head, 512-col q-block): 16 score steps per unit; each step is
    one [64-deep] QK matmul pair (parities in the two PE row halves), one
    exp instruction (ScalarE table-exp for 12 steps, VectorE Schraudolph
    int16->bf16 for 4), plus TWO PV-chain matmuls of the PREVIOUS unit
    threaded between - so ScalarE never idles at unit boundaries.
  - PV: full-128-deep accumulation chains (one PSUM bank per col-group,
    LDWEIGHTS hides in the background weight buffer).
  - Output: PE transpose -> DVE PSUM->SBUF copy -> gpsimd normalize_recip
    (the only engine with slack) -> DMA store.

The q columns are processed in a (parity, block) interleaved order; the
output store APs undo the permutation (col-group (b, parity p, sub t)
holds global q with q mod 16 == 8b + 2t + p).
"""

import numpy as np

import concourse.bass as bass
import concourse.mybir as mybir
import concourse.tile as tile
from concourse import bacc
from concourse.bass_utils import run_bass_kernel_spmd
from concourse.masks import make_identity

B, H, S, D = 2, 16, 2048, 64
N_CORES = 8
HPC = (B * H) // N_CORES
P = 128
NJ = S // P
NG = NJ // 2
QB = 512
LAG = 4
SCALE = 1.0 / float(D) ** 0.5
F32 = mybir.dt.float32
BF16 = mybir.dt.bfloat16
I16 = mybir.dt.int16

LOG2E = 1.4426950408889634
A16 = float((1 << 7) * LOG2E * SCALE)
B16 = float(127.0 * (1 << 7) - 7.42)

# exp steps (tt, jj) run on the DVE via Schraudolph; listing both tt of a
# jj approximates chunks {2jj, 2jj+1} fully.
DVE_TILES = ((0, 1), (0, 3), (0, 5), (0, 7), (1, 1), (1, 3), (1, 5))

_CACHED = {}
DEFAULT_CFG = {}


def _build_module(**cfg):
    nc = bacc.Bacc(None)
    q = nc.dram_tensor("q", [HPC, S, D], F32, kind="ExternalInput")
    k = nc.dram_tensor("k", [S, D], F32, kind="ExternalInput")
    v = nc.dram_tensor("v", [S, D], F32, kind="ExternalInput")
    o = nc.dram_tensor("o", [HPC, S, D], F32, kind="ExternalOutput")
    _trace_body(nc, q, k, v, o, **cfg)
    nc.compile()
    return nc


def _trace_body(nc, q, k, v, o, sg_bufs=3, pv_bufs=2, pt_bufs=3, dve_tiles=None):
    dve_tiles = DVE_TILES if dve_tiles is None else dve_tiles
    with tile.TileContext(nc) as tc:
        with (
            tc.tile_pool(name="const", bufs=1) as cpool,
            tc.tile_pool(name="qb", bufs=2) as qpool,
            tc.tile_pool(name="ptb", bufs=pt_bufs) as ptpool,
            tc.tile_pool(name="wk", bufs=2) as wpool,
            tc.tile_pool(name="sg", bufs=sg_bufs, space="PSUM") as sgpool,
            tc.tile_pool(name="pv", bufs=pv_bufs, space="PSUM") as pvpool,
        ):
            # ---- prologue v4: HWDGE fp32 loads + engine casts; kT/qT(h0)
            # transposed on the (otherwise idle) PE through PSUM slots the sg
            # pool will reuse. This keeps the Scalar queue free of 1.2us DMA
            # transposes so exp can start with the first score matmul. ----
            identity = cpool.tile([P, P], F32)
            make_identity(nc, identity)
            identity_bf = cpool.tile([P, P], BF16)
            nc.vector.tensor_copy(identity_bf[:], identity[:])

            k_src = k.rearrange("(p c) d -> p c d", p=P)
            k_nat = cpool.tile([P, NJ, D], F32)
            nc.sync.dma_start(k_nat[:, 0 : NJ // 2, :], k_src[:, 0 : NJ // 2, :])
            nc.sync.dma_start(k_nat[:, NJ // 2 :, :], k_src[:, NJ // 2 :, :])
            k_bf = cpool.tile([P, NJ, D], BF16)
            nc.scalar.activation(
                k_bf[:, 0 : NJ // 2, :], k_nat[:, 0 : NJ // 2, :],
                mybir.ActivationFunctionType.Copy,
            )
            nc.vector.tensor_copy(k_bf[:, NJ // 2 :, :], k_nat[:, NJ // 2 :, :])

            q0_src = q[0].rearrange("(p c) d -> p c d", p=P)
            q0_nat = cpool.tile([P, NJ, D], F32)
            nc.scalar.dma_start(q0_nat[:, 0 : NJ // 2, :], q0_src[:, 0 : NJ // 2, :])
            nc.scalar.dma_start(q0_nat[:, NJ // 2 :, :], q0_src[:, NJ // 2 :, :])

            v_nat = cpool.tile([P, NJ, D], F32)
            nc.sync.dma_start(v_nat[:], v.rearrange("(p c) d -> p c d", p=P))

            def transpose_to(dst_sb, src_bf, psname):
                ps = sgpool.tile(
                    [P, NG, P], BF16, tag="sg", name=psname,
                    padded_shape=[P, 2 * NG, P],
                )
                for g in range(NG):
                    nc.tensor.transpose(
                        ps[:, g, :], src_bf[:, 2 * g : 2 * g + 2, :], identity_bf[:]
                    )
                nc.scalar.activation(
                    dst_sb[:, 0 : NG // 2, :], ps[:, 0 : NG // 2, :],
                    mybir.ActivationFunctionType.Copy,
                )
                nc.vector.tensor_copy(dst_sb[:, NG // 2 :, :], ps[:, NG // 2 :, :])

            kTs = cpool.tile([P, NG, P], BF16)
            transpose_to(kTs, k_bf, "kt_ps")
            kTs2 = cpool.tile([P, NG, P], BF16)
            nc.gpsimd.dma_start(kTs2[0:64, :, :], kTs[64:P, :, :])
            nc.gpsimd.dma_start(kTs2[64:P, :, :], kTs[0:64, :, :])

            vp = cpool.tile([P, NJ, D + 1], BF16)
            nc.gpsimd.memset(vp[:, :, D], 1.0)
            nc.vector.tensor_copy(vp[:, :, 0:D], v_nat[:])

            def load_qT(h):
                q_bf = qpool.tile([P, NJ, D], BF16, tag="qbf", name=f"qbf{h}")
                q_src = q[h].rearrange("(p c) d -> p c d", p=P)
                if h == 0:
                    nc.vector.tensor_copy(q_bf[:, 0 : NJ // 2, :], q0_nat[:, 0 : NJ // 2, :])
                    nc.scalar.activation(
                        q_bf[:, NJ // 2 :, :], q0_nat[:, NJ // 2 :, :],
                        mybir.ActivationFunctionType.Copy,
                    )
                else:
                    nc.gpsimd.dma_start(q_bf[:, 0 : NJ // 2, :], q_src[:, 0 : NJ // 2, :])
                    nc.gpsimd.dma_start(q_bf[:, NJ // 2 :, :], q_src[:, NJ // 2 :, :])
                qTs = qpool.tile([P, NG, P], BF16, tag="qTs", name=f"qTs{h}")
                if h == 0:
                    transpose_to(qTs, q_bf, "qt_ps0")
                else:
                    for g in range(NG):
                        nc.sync.dma_start_transpose(
                            qTs[:, g, :], q_bf[:, 2 * g : 2 * g + 2, :]
                        )
                return qTs

            def pt_idx(p, c):
                jj, jp = divmod(c, 2)
                if p == 0:
                    return (0, jj, 0) if jp == 0 else (1, jj, 0)
                return (0, jj, 1) if jp == 1 else (1, jj, 1)

            qTs_cur = load_qT(0)

            # ---- steady-state pipeline over units u = (h, b); PV of unit
            # u-1 threads through unit u's score steps ----
            units = [(h, b) for h in range(HPC) for b in range(2)]
            qTs_of = {0: qTs_cur}
            pT_of = {}
            pv_of = {}

            def pt_idx(p, c):
                jj, jp = divmod(c, 2)
                if p == 0:
                    return (0, jj, 0) if jp == 0 else (1, jj, 0)
                return (0, jj, 1) if jp == 1 else (1, jj, 1)

            def emit_output(h, b, p, pv):
                otr = pvpool.tile(
                    [P, 4, D + 1], BF16, tag="pv", name=f"otr{h}_{b}_{p}",
                    padded_shape=[P, 4, QB // 2],
                )
                oev = wpool.tile([D + 1, QB], BF16, tag="oev", name=f"oev{h}_{b}_{p}")
                nc.vector.tensor_copy(oev[:], pv[:])
                for t in range(4):
                    nc.tensor.transpose(
                        otr[:, t, :],
                        oev[:, P * t : P * (t + 1)],
                        identity_bf[0 : D + 1, 0 : D + 1],
                    )
                osb = wpool.tile([P, 4, D + 1], F32, tag="osb", name=f"osb{h}_{b}_{p}")
                nc.scalar.activation(
                    osb[:], otr[:], mybir.ActivationFunctionType.Copy
                )
                oout = wpool.tile([P, 4, D], F32, tag="oout", name=f"oout{h}_{b}_{p}")
                for t in range(4):
                    nc.gpsimd.normalize_recip(
                        oout[:, t, :], osb[:, t, 0:D], osb[:, t, D : D + 1]
                    )
                nc.sync.dma_start(
                    o[h].rearrange("(p c) d -> p c d", p=P)[
                        :, 8 * b + p : 8 * b + 8 : 2, :
                    ],
                    oout[:],
                )

            def emit_pv_steps(u_prev, s):
                h, b = units[u_prev]
                p = s // 8
                pT = pT_of[u_prev]
                if s % 8 == 0:
                    pv_of[u_prev * 2 + p] = pvpool.tile(
                        [D + 1, QB], F32, tag="pv", name=f"pv{h}_{b}_{p}",
                        padded_shape=[P, QB],
                    )
                pv = pv_of[u_prev * 2 + p]
                for c in (2 * (s % 8), 2 * (s % 8) + 1):
                    tt, jj, sl = pt_idx(p, c)
                    nc.tensor.matmul(
                        pv[:],
                        lhsT=vp[:, c, :],
                        rhs=pT[:, tt, jj, sl, :],
                        start=(c == 0),
                        stop=(c == NJ - 1),
                        skip_group_check=True,
                    )
                if s % 8 == 7:
                    emit_output(h, b, p, pv)

            def emit_exp(pT, sg, tt, jj):
                out_ap = pT[:, tt, jj, :, :]
                if (tt, jj) in dve_tiles:
                    nc.vector.tensor_scalar(
                        out_ap.bitcast(I16),
                        sg[:],
                        A16,
                        B16,
                        mybir.AluOpType.mult,
                        mybir.AluOpType.add,
                    )
                else:
                    nc.scalar.activation(
                        out_ap,
                        sg[:],
                        mybir.ActivationFunctionType.Exp,
                        scale=SCALE,
                    )

            # 2-step phases: [score s0, score s1] then [pv x4] of the previous
            # unit - halves the 64-row-tile <-> 128 mode switches on the PE.
            for u, (h, b) in enumerate(units):
                if b == 0:
                    qTs = qTs_of[h]
                pT = ptpool.tile([P, 2, NG, 2, QB], BF16, tag="pT", name=f"pT{h}_{b}")
                pT_of[u] = pT
                for sp in range(8):
                    sgs = []
                    for s in (2 * sp, 2 * sp + 1):
                        jj, tt = s % 8, s // 8
                        kt_src = kTs if tt == 0 else kTs2
                        sg = sgpool.tile(
                            [P, 2, QB], F32, tag="sg", name=f"sg{h}_{b}_{s}"
                        )
                        sgs.append((sg, tt, jj))
                        nc.tensor.matmul(
                            sg[:, 0, :],
                            lhsT=kt_src[0:64, jj, :],
                            rhs=qTs[0:64, 4 * b : 4 * (b + 1), :],
                            start=True, stop=True,
                        )
                        nc.tensor.matmul(
                            sg[:, 1, :],
                            lhsT=kt_src[64:P, jj, :],
                            rhs=qTs[64:P, 4 * b : 4 * (b + 1), :],
                            start=True, stop=True,
                        )
                    for i, s in enumerate((2 * sp, 2 * sp + 1)):
                        if u > 0:
                            emit_pv_steps(u - 1, s)
                        emit_exp(pT, *sgs[i])
                    if sp == 1 and b == 1 and h + 1 < HPC:
                        qTs_of[h + 1] = load_qT(h + 1)

            for s in range(16):
                emit_pv_steps(len(units) - 1, s)


def _get_module(**cfg):
    key = tuple(sorted(cfg.items()))
    if key not in _CACHED:
        _CACHED[key] = _build_module(**cfg)
    return _CACHED[key]


def make_in_maps(Q, K, V):
    Q = np.asarray(Q, dtype=np.float32)
    K = np.asarray(K, dtype=np.float32)
    V = np.asarray(V, dtype=np.float32)
    in_maps = []
    for c in range(N_CORES):
        b = c // (N_CORES // B)
        h0 = HPC * (c % (N_CORES // B))
        in_maps.append(
            {
                "q": np.ascontiguousarray(Q[b, h0 : h0 + HPC]),
                "k": np.ascontiguousarray(K[b, 0]),
                "v": np.ascontiguousarray(V[b, 0]),
            }
        )
    return in_maps


def assemble_output(results):
    out = np.empty((B, H, S, D), dtype=np.float32)
    for c in range(N_CORES):
        b = c // (N_CORES // B)
        h0 = HPC * (c % (N_CORES // B))
        out[b, h0 : h0 + HPC] = results[c]["o"]
    return out


def kernel(Q, K, V):
    nc = _get_module(**DEFAULT_CFG)
    res = run_bass_kernel_spmd(nc, make_in_maps(Q, K, V), core_ids=list(range(N_CORES)))
    return assemble_output(res.results)



# revision 13
# speedup vs baseline: 1.1274x; 1.0439x over previous
"""MQA attention kernel v3 for Trainium2 (8 NeuronCores, Bass/Tile).

Problem: Q [2,16,2048,64], K/V [2,1,2048,64] fp32, out = softmax(QK^T/8) V.
Sharding: 32 (batch, head) pairs over 8 cores -> 4 heads/core; one batch's
K/V per core.

v3 = v2's data path with a software-pipelined emission order:
  - bf16 everywhere; Q^T/K^T via gpsimd DMA-cast + xbar DMA transposes of
    [128,128] chunk-pairs (zero PE/DVE cost). kTs + half-swapped kTs2 give
    every chunk in both partition halves.
  - Unit = (head, 512-col q-block): 16 score steps per unit; each step is
    one [64-deep] QK matmul pair (parities in the two PE row halves), one
    exp instruction (ScalarE table-exp for 12 steps, VectorE Schraudolph
    int16->bf16 for 4), plus TWO PV-chain matmuls of the PREVIOUS unit
    threaded between - so ScalarE never idles at unit boundaries.
  - PV: full-128-deep accumulation chains (one PSUM bank per col-group,
    LDWEIGHTS hides in the background weight buffer).
  - Output: PE transpose -> DVE PSUM->SBUF copy -> gpsimd normalize_recip
    (the only engine with slack) -> DMA store.

The q columns are processed in a (parity, block) interleaved order; the
output store APs undo the permutation (col-group (b, parity p, sub t)
holds global q with q mod 16 == 8b + 2t + p).
"""

import numpy as np

import concourse.bass as bass
import concourse.mybir as mybir
import concourse.tile as tile
from concourse import bacc
from concourse.bass_utils import run_bass_kernel_spmd
from concourse.masks import make_identity

B, H, S, D = 2, 16, 2048, 64
N_CORES = 8
HPC = (B * H) // N_CORES
P = 128
NJ = S // P
NG = NJ // 2
QB = 512
LAG = 4
SCALE = 1.0 / float(D) ** 0.5
F32 = mybir.dt.float32
BF16 = mybir.dt.bfloat16
I16 = mybir.dt.int16

LOG2E = 1.4426950408889634
A16 = float((1 << 7) * LOG2E * SCALE)
B16 = float(127.0 * (1 << 7) - 7.42)

# exp steps (tt, jj) run on the DVE via Schraudolph; listing both tt of a
# jj approximates chunks {2jj, 2jj+1} fully.
DVE_TILES = ((0, 1), (0, 3), (0, 5), (0, 7), (1, 1), (1, 3), (1, 5))

_CACHED = {}
DEFAULT_CFG = {}


def _build_module(**cfg):
    nc = bacc.Bacc(None)
    q = nc.dram_tensor("q", [HPC, S, D], F32, kind="ExternalInput")
    k = nc.dram_tensor("k", [S, D], F32, kind="ExternalInput")
    v = nc.dram_tensor("v", [S, D], F32, kind="ExternalInput")
    o = nc.dram_tensor("o", [HPC, S, D], F32, kind="ExternalOutput")
    _trace_body(nc, q, k, v, o, **cfg)
    nc.compile()
    return nc


def _trace_body(nc, q, k, v, o, sg_bufs=3, pv_bufs=2, pt_bufs=3, dve_tiles=None):
    dve_tiles = DVE_TILES if dve_tiles is None else dve_tiles
    with tile.TileContext(nc) as tc:
        with (
            tc.tile_pool(name="const", bufs=1) as cpool,
            tc.tile_pool(name="qb", bufs=2) as qpool,
            tc.tile_pool(name="ptb", bufs=pt_bufs) as ptpool,
            tc.tile_pool(name="wk", bufs=2) as wpool,
            tc.tile_pool(name="sg", bufs=sg_bufs, space="PSUM") as sgpool,
            tc.tile_pool(name="pv", bufs=pv_bufs, space="PSUM") as pvpool,
        ):
            # ---- prologue v4: HWDGE fp32 loads + engine casts; kT/qT(h0)
            # transposed on the (otherwise idle) PE through PSUM slots the sg
            # pool will reuse. This keeps the Scalar queue free of 1.2us DMA
            # transposes so exp can start with the first score matmul. ----
            identity = cpool.tile([P, P], F32)
            make_identity(nc, identity)
            identity_bf = cpool.tile([P, P], BF16)
            nc.vector.tensor_copy(identity_bf[:], identity[:])

            k_src = k.rearrange("(p c) d -> p c d", p=P)
            k_nat = cpool.tile([P, NJ, D], F32)
            nc.sync.dma_start(k_nat[:, 0 : NJ // 2, :], k_src[:, 0 : NJ // 2, :])
            nc.sync.dma_start(k_nat[:, NJ // 2 :, :], k_src[:, NJ // 2 :, :])
            k_bf = cpool.tile([P, NJ, D], BF16)
            nc.scalar.activation(
                k_bf[:, 0 : NJ // 2, :], k_nat[:, 0 : NJ // 2, :],
                mybir.ActivationFunctionType.Copy,
            )
            nc.vector.tensor_copy(k_bf[:, NJ // 2 :, :], k_nat[:, NJ // 2 :, :])
            # chunk-pair-swapped copy of k_bf: chunk 2g <-> 2g+1, so kTs2 can
            # be built by plain PE transposes (no SBUF->SBUF swap DMA).
            k_bf2 = cpool.tile([P, NJ, D], BF16)
            nc.scalar.activation(
                k_bf2[:, 0 : NJ : 2, :], k_nat[:, 1 : NJ : 2, :],
                mybir.ActivationFunctionType.Copy,
            )
            nc.vector.tensor_copy(k_bf2[:, 1 : NJ : 2, :], k_nat[:, 0 : NJ : 2, :])

            q0_src = q[0].rearrange("(p c) d -> p c d", p=P)
            q0_nat = cpool.tile([P, NJ, D], F32)
            nc.sync.dma_start(q0_nat[:, 0 : NJ // 2, :], q0_src[:, 0 : NJ // 2, :])
            nc.sync.dma_start(q0_nat[:, NJ // 2 :, :], q0_src[:, NJ // 2 :, :])

            v_nat = cpool.tile([P, NJ, D], F32)
            nc.scalar.dma_start(v_nat[:], v.rearrange("(p c) d -> p c d", p=P))

            def transpose_to(dst_sb, src_bf, psname):
                ps = sgpool.tile(
                    [P, NG, P], BF16, tag="sg", name=psname,
                    padded_shape=[P, 2 * NG, P],
                )
                for g in range(NG):
                    nc.tensor.transpose(
                        ps[:, g, :], src_bf[:, 2 * g : 2 * g + 2, :], identity_bf[:]
                    )
                nc.scalar.activation(
                    dst_sb[:, 0 : NG // 2, :], ps[:, 0 : NG // 2, :],
                    mybir.ActivationFunctionType.Copy,
                )
                nc.vector.tensor_copy(dst_sb[:, NG // 2 :, :], ps[:, NG // 2 :, :])

            kTs = cpool.tile([P, NG, P], BF16)
            transpose_to(kTs, k_bf, "kt_ps")
            kTs2 = cpool.tile([P, NG, P], BF16)
            transpose_to(kTs2, k_bf2, "kt2_ps")

            vp = cpool.tile([P, NJ, D + 1], BF16)
            nc.gpsimd.memset(vp[:, :, D], 1.0)
            nc.vector.tensor_copy(vp[:, :, 0:D], v_nat[:])

            def load_qT(h):
                q_bf = qpool.tile([P, NJ, D], BF16, tag="qbf", name=f"qbf{h}")
                q_src = q[h].rearrange("(p c) d -> p c d", p=P)
                if h == 0:
                    nc.vector.tensor_copy(q_bf[:, 0 : NJ // 2, :], q0_nat[:, 0 : NJ // 2, :])
                    nc.scalar.activation(
                        q_bf[:, NJ // 2 :, :], q0_nat[:, NJ // 2 :, :],
                        mybir.ActivationFunctionType.Copy,
                    )
                else:
                    nc.gpsimd.dma_start(q_bf[:, 0 : NJ // 2, :], q_src[:, 0 : NJ // 2, :])
                    nc.gpsimd.dma_start(q_bf[:, NJ // 2 :, :], q_src[:, NJ // 2 :, :])
                qTs = qpool.tile([P, NG, P], BF16, tag="qTs", name=f"qTs{h}")
                if h == 0:
                    transpose_to(qTs, q_bf, "qt_ps0")
                else:
                    for g in range(NG):
                        nc.sync.dma_start_transpose(
                            qTs[:, g, :], q_bf[:, 2 * g : 2 * g + 2, :]
                        )
                return qTs

            def pt_idx(p, c):
                jj, jp = divmod(c, 2)
                if p == 0:
                    return (0, jj, 0) if jp == 0 else (1, jj, 0)
                return (0, jj, 1) if jp == 1 else (1, jj, 1)

            qTs_cur = load_qT(0)

            # ---- steady-state pipeline over units u = (h, b); PV of unit
            # u-1 threads through unit u's score steps ----
            units = [(h, b) for h in range(HPC) for b in range(2)]
            qTs_of = {0: qTs_cur}
            pT_of = {}
            pv_of = {}

            def pt_idx(p, c):
                jj, jp = divmod(c, 2)
                if p == 0:
                    return (0, jj, 0) if jp == 0 else (1, jj, 0)
                return (0, jj, 1) if jp == 1 else (1, jj, 1)

            def emit_output(h, b, p, pv):
                otr = pvpool.tile(
                    [P, 4, D + 1], BF16, tag="pv", name=f"otr{h}_{b}_{p}",
                    padded_shape=[P, 4, QB // 2],
                )
                oev = wpool.tile([D + 1, QB], BF16, tag="oev", name=f"oev{h}_{b}_{p}")
                nc.vector.tensor_copy(oev[:], pv[:])
                for t in range(4):
                    nc.tensor.transpose(
                        otr[:, t, :],
                        oev[:, P * t : P * (t + 1)],
                        identity_bf[0 : D + 1, 0 : D + 1],
                    )
                osb = wpool.tile([P, 4, D + 1], F32, tag="osb", name=f"osb{h}_{b}_{p}")
                nc.scalar.activation(
                    osb[:], otr[:], mybir.ActivationFunctionType.Copy
                )
                oout = wpool.tile([P, 4, D], F32, tag="oout", name=f"oout{h}_{b}_{p}")
                for t in range(4):
                    nc.gpsimd.normalize_recip(
                        oout[:, t, :], osb[:, t, 0:D], osb[:, t, D : D + 1]
                    )
                nc.sync.dma_start(
                    o[h].rearrange("(p c) d -> p c d", p=P)[
                        :, 8 * b + p : 8 * b + 8 : 2, :
                    ],
                    oout[:],
                )

            def emit_pv_steps(u_prev, s):
                h, b = units[u_prev]
                p = s // 8
                pT = pT_of[u_prev]
                if s % 8 == 0:
                    pv_of[u_prev * 2 + p] = pvpool.tile(
                        [D + 1, QB], F32, tag="pv", name=f"pv{h}_{b}_{p}",
                        padded_shape=[P, QB],
                    )
                pv = pv_of[u_prev * 2 + p]
                for c in (2 * (s % 8), 2 * (s % 8) + 1):
                    tt, jj, sl = pt_idx(p, c)
                    nc.tensor.matmul(
                        pv[:],
                        lhsT=vp[:, c, :],
                        rhs=pT[:, tt, jj, sl, :],
                        start=(c == 0),
                        stop=(c == NJ - 1),
                        skip_group_check=True,
                    )
                if s % 8 == 7:
                    emit_output(h, b, p, pv)

            def emit_exp(pT, sg, tt, jj):
                out_ap = pT[:, tt, jj, :, :]
                if (tt, jj) in dve_tiles:
                    nc.vector.tensor_scalar(
                        out_ap.bitcast(I16),
                        sg[:],
                        A16,
                        B16,
                        mybir.AluOpType.mult,
                        mybir.AluOpType.add,
                    )
                else:
                    nc.scalar.activation(
                        out_ap,
                        sg[:],
                        mybir.ActivationFunctionType.Exp,
                        scale=SCALE,
                    )

            for u, (h, b) in enumerate(units):
                if b == 0:
                    qTs = qTs_of[h]
                pT = ptpool.tile([P, 2, NG, 2, QB], BF16, tag="pT", name=f"pT{h}_{b}")
                pT_of[u] = pT
                for s in range(16):
                    jj, tt = s % 8, s // 8
                    kt_src = kTs if tt == 0 else kTs2
                    sg = sgpool.tile([P, 2, QB], F32, tag="sg", name=f"sg{h}_{b}_{s}")
                    nc.tensor.matmul(
                        sg[:, 0, :],
                        lhsT=kt_src[0:64, jj, :],
                        rhs=qTs[0:64, 4 * b : 4 * (b + 1), :],
                        start=True, stop=True,
                    )
                    nc.tensor.matmul(
                        sg[:, 1, :],
                        lhsT=kt_src[64:P, jj, :],
                        rhs=qTs[64:P, 4 * b : 4 * (b + 1), :],
                        start=True, stop=True,
                    )
                    if u > 0:
                        emit_pv_steps(u - 1, s)
                    emit_exp(pT, sg, tt, jj)
                    if s == 3 and b == 1 and h + 1 < HPC:
                        qTs_of[h + 1] = load_qT(h + 1)

            for s in range(16):
                emit_pv_steps(len(units) - 1, s)


def _get_module(**cfg):
    key = tuple(sorted(cfg.items()))
    if key not in _CACHED:
        _CACHED[key] = _build_module(**cfg)
    return _CACHED[key]


def make_in_maps(Q, K, V):
    Q = np.asarray(Q, dtype=np.float32)
    K = np.asarray(K, dtype=np.float32)
    V = np.asarray(V, dtype=np.float32)
    in_maps = []
    for c in range(N_CORES):
        b = c // (N_CORES // B)
        h0 = HPC * (c % (N_CORES // B))
        in_maps.append(
            {
                "q": np.ascontiguousarray(Q[b, h0 : h0 + HPC]),
                "k": np.ascontiguousarray(K[b, 0]),
                "v": np.ascontiguousarray(V[b, 0]),
            }
        )
    return in_maps


def assemble_output(results):
    out = np.empty((B, H, S, D), dtype=np.float32)
    for c in range(N_CORES):
        b = c // (N_CORES // B)
        h0 = HPC * (c % (N_CORES // B))
        out[b, h0 : h0 + HPC] = results[c]["o"]
    return out


def kernel(Q, K, V):
    nc = _get_module(**DEFAULT_CFG)
    res = run_bass_kernel_spmd(nc, make_in_maps(Q, K, V), core_ids=list(range(N_CORES)))
    return assemble_output(res.results)



# revision 17
# speedup vs baseline: 1.1291x; 1.0014x over previous
"""MQA attention kernel v3 for Trainium2 (8 NeuronCores, Bass/Tile).

Problem: Q [2,16,2048,64], K/V [2,1,2048,64] fp32, out = softmax(QK^T/8) V.
Sharding: 32 (batch, head) pairs over 8 cores -> 4 heads/core; one batch's
K/V per core.

v3 = v2's data path with a software-pipelined emission order:
  - bf16 everywhere; Q^T/K^T via gpsimd DMA-cast + xbar DMA transposes of
    [128,128] chunk-pairs (zero PE/DVE cost). kTs + half-swapped kTs2 give
    every chunk in both partition halves.
  - Unit = (head, 512-col q-block): 16 score steps per unit; each step is
    one [64-deep] QK matmul pair (parities in the two PE row halves), one
    exp instruction (ScalarE table-exp for 12 steps, VectorE Schraudolph
    int16->bf16 for 4), plus TWO PV-chain matmuls of the PREVIOUS unit
    threaded between - so ScalarE never idles at unit boundaries.
  - PV: full-128-deep accumulation chains (one PSUM bank per col-group,
    LDWEIGHTS hides in the background weight buffer).
  - Output: PE transpose -> DVE PSUM->SBUF copy -> gpsimd normalize_recip
    (the only engine with slack) -> DMA store.

The q columns are processed in a (parity, block) interleaved order; the
output store APs undo the permutation (col-group (b, parity p, sub t)
holds global q with q mod 16 == 8b + 2t + p).
"""

import numpy as np

import concourse.bass as bass
import concourse.mybir as mybir
import concourse.tile as tile
from concourse import bacc
from concourse.bass_utils import run_bass_kernel_spmd
from concourse.masks import make_identity

B, H, S, D = 2, 16, 2048, 64
N_CORES = 8
HPC = (B * H) // N_CORES
P = 128
NJ = S // P
NG = NJ // 2
QB = 512
LAG = 4
SCALE = 1.0 / float(D) ** 0.5
F32 = mybir.dt.float32
BF16 = mybir.dt.bfloat16
I16 = mybir.dt.int16

LOG2E = 1.4426950408889634
A16 = float((1 << 7) * LOG2E * SCALE)
B16 = float(127.0 * (1 << 7) - 7.42)

# exp steps (tt, jj) run on the DVE via Schraudolph; listing both tt of a
# jj approximates chunks {2jj, 2jj+1} fully.
# steps {1,2,3,5,10,11,13} - kept away from col-group boundary steps
# (7, 8, 15, 0) where the DVE also carries output-evacuation copies.
DVE_TILES = ((0, 1), (0, 2), (0, 3), (0, 5), (1, 2), (1, 3), (1, 5))

_CACHED = {}
DEFAULT_CFG = {}


def _build_module(**cfg):
    nc = bacc.Bacc(None)
    q = nc.dram_tensor("q", [HPC, S, D], F32, kind="ExternalInput")
    k = nc.dram_tensor("k", [S, D], F32, kind="ExternalInput")
    v = nc.dram_tensor("v", [S, D], F32, kind="ExternalInput")
    o = nc.dram_tensor("o", [HPC, S, D], F32, kind="ExternalOutput")
    _trace_body(nc, q, k, v, o, **cfg)
    nc.compile()
    return nc


def _trace_body(nc, q, k, v, o, sg_bufs=3, pv_bufs=2, pt_bufs=3, dve_tiles=None):
    dve_tiles = DVE_TILES if dve_tiles is None else dve_tiles
    with tile.TileContext(nc) as tc:
        with (
            tc.tile_pool(name="const", bufs=1) as cpool,
            tc.tile_pool(name="qb", bufs=2) as qpool,
            tc.tile_pool(name="ptb", bufs=pt_bufs) as ptpool,
            tc.tile_pool(name="wk", bufs=2) as wpool,
            tc.tile_pool(name="sg", bufs=sg_bufs, space="PSUM") as sgpool,
            tc.tile_pool(name="pv", bufs=pv_bufs, space="PSUM") as pvpool,
        ):
            # ---- prologue v4: HWDGE fp32 loads + engine casts; kT/qT(h0)
            # transposed on the (otherwise idle) PE through PSUM slots the sg
            # pool will reuse. This keeps the Scalar queue free of 1.2us DMA
            # transposes so exp can start with the first score matmul. ----
            identity = cpool.tile([P, P], F32)
            make_identity(nc, identity)
            identity_bf = cpool.tile([P, P], BF16)
            nc.vector.tensor_copy(identity_bf[:], identity[:])

            # K straight to bf16 on the (otherwise idle) gpsimd SW-DGE rings:
            # both the natural chunk order and the pair-swapped order (chunk
            # 2g <-> 2g+1), so kTs and kTs2 are plain PE transposes away.
            k_src = k.rearrange("(p c) d -> p c d", p=P)
            k_bf = cpool.tile([P, NJ, D], BF16)
            nc.gpsimd.dma_start(k_bf[:, 0 : NJ // 2, :], k_src[:, 0 : NJ // 2, :])
            nc.gpsimd.dma_start(k_bf[:, NJ // 2 :, :], k_src[:, NJ // 2 :, :])
            k_bf2 = cpool.tile([P, NJ, D], BF16)
            nc.gpsimd.dma_start(k_bf2[:, 0 : NJ : 2, :], k_src[:, 1 : NJ : 2, :])
            nc.gpsimd.dma_start(k_bf2[:, 1 : NJ : 2, :], k_src[:, 0 : NJ : 2, :])

            q0_src = q[0].rearrange("(p c) d -> p c d", p=P)
            q0_nat = cpool.tile([P, NJ, D], F32)
            nc.sync.dma_start(q0_nat[:, 0 : NJ // 2, :], q0_src[:, 0 : NJ // 2, :])
            nc.sync.dma_start(q0_nat[:, NJ // 2 :, :], q0_src[:, NJ // 2 :, :])

            v_nat = cpool.tile([P, NJ, D], F32)
            nc.scalar.dma_start(v_nat[:], v.rearrange("(p c) d -> p c d", p=P))

            def transpose_to(dst_sb, src_bf, psname):
                ps = sgpool.tile(
                    [P, NG, P], BF16, tag="sg", name=psname,
                    padded_shape=[P, 2 * NG, P],
                )
                for g in range(NG):
                    nc.tensor.transpose(
                        ps[:, g, :], src_bf[:, 2 * g : 2 * g + 2, :], identity_bf[:]
                    )
                nc.scalar.activation(
                    dst_sb[:, 0 : NG // 2, :], ps[:, 0 : NG // 2, :],
                    mybir.ActivationFunctionType.Copy,
                )
                nc.vector.tensor_copy(dst_sb[:, NG // 2 :, :], ps[:, NG // 2 :, :])

            kTs = cpool.tile([P, NG, P], BF16)
            transpose_to(kTs, k_bf, "kt_ps")
            kTs2 = cpool.tile([P, NG, P], BF16)
            transpose_to(kTs2, k_bf2, "kt2_ps")

            vp = cpool.tile([P, NJ, D + 1], BF16)
            nc.gpsimd.memset(vp[:, :, D], 1.0)
            nc.vector.tensor_copy(vp[:, :, 0:D], v_nat[:])

            def load_qT(h):
                q_bf = qpool.tile([P, NJ, D], BF16, tag="qbf", name=f"qbf{h}")
                q_src = q[h].rearrange("(p c) d -> p c d", p=P)
                if h == 0:
                    nc.vector.tensor_copy(q_bf[:, 0 : NJ // 2, :], q0_nat[:, 0 : NJ // 2, :])
                    nc.scalar.activation(
                        q_bf[:, NJ // 2 :, :], q0_nat[:, NJ // 2 :, :],
                        mybir.ActivationFunctionType.Copy,
                    )
                else:
                    nc.gpsimd.dma_start(q_bf[:, 0 : NJ // 2, :], q_src[:, 0 : NJ // 2, :])
                    nc.gpsimd.dma_start(q_bf[:, NJ // 2 :, :], q_src[:, NJ // 2 :, :])
                qTs = qpool.tile([P, NG, P], BF16, tag="qTs", name=f"qTs{h}")
                if h == 0:
                    transpose_to(qTs, q_bf, "qt_ps0")
                else:
                    for g in range(NG):
                        nc.sync.dma_start_transpose(
                            qTs[:, g, :], q_bf[:, 2 * g : 2 * g + 2, :]
                        )
                return qTs

            def pt_idx(p, c):
                jj, jp = divmod(c, 2)
                if p == 0:
                    return (0, jj, 0) if jp == 0 else (1, jj, 0)
                return (0, jj, 1) if jp == 1 else (1, jj, 1)

            qTs_cur = load_qT(0)

            # ---- steady-state pipeline over units u = (h, b); PV of unit
            # u-1 threads through unit u's score steps ----
            units = [(h, b) for h in range(HPC) for b in range(2)]
            qTs_of = {0: qTs_cur}
            pT_of = {}
            pv_of = {}

            def pt_idx(p, c):
                jj, jp = divmod(c, 2)
                if p == 0:
                    return (0, jj, 0) if jp == 0 else (1, jj, 0)
                return (0, jj, 1) if jp == 1 else (1, jj, 1)

            def emit_output(h, b, p, pv, fast_tail=False):
                otr = pvpool.tile(
                    [P, 4, D + 1], BF16, tag="pv", name=f"otr{h}_{b}_{p}",
                    padded_shape=[P, 4, QB // 2],
                )
                oev = wpool.tile([D + 1, QB], BF16, tag="oev", name=f"oev{h}_{b}_{p}")
                nc.vector.tensor_copy(oev[:], pv[:])
                for t in range(4):
                    nc.tensor.transpose(
                        otr[:, t, :],
                        oev[:, P * t : P * (t + 1)],
                        identity_bf[0 : D + 1, 0 : D + 1],
                    )
                osb = wpool.tile([P, 4, D + 1], F32, tag="osb", name=f"osb{h}_{b}_{p}")
                nc.scalar.activation(
                    osb[:], otr[:], mybir.ActivationFunctionType.Copy
                )
                oout = wpool.tile([P, 4, D], F32, tag="oout", name=f"oout{h}_{b}_{p}")
                if fast_tail:
                    # drain phase: gpsimd would serialize; use DVE recip +
                    # ACT per-partition scale instead.
                    rec = wpool.tile([P, 4], F32, tag="rec", name=f"rec{h}_{b}_{p}")
                    nc.vector.reciprocal(rec[:], osb[:, :, D])
                    for t in range(4):
                        nc.scalar.activation(
                            oout[:, t, :], osb[:, t, 0:D],
                            mybir.ActivationFunctionType.Copy,
                            scale=rec[:, t : t + 1],
                        )
                else:
                    for t in range(4):
                        nc.gpsimd.normalize_recip(
                            oout[:, t, :], osb[:, t, 0:D], osb[:, t, D : D + 1]
                        )
                nc.sync.dma_start(
                    o[h].rearrange("(p c) d -> p c d", p=P)[
                        :, 8 * b + p : 8 * b + 8 : 2, :
                    ],
                    oout[:],
                )

            def emit_pv_chunks(u_prev, p, chunks, alloc=False, close=False):
                h, b = units[u_prev]
                pT = pT_of[u_prev]
                if alloc:
                    pv_of[u_prev * 2 + p] = pvpool.tile(
                        [D + 1, QB], F32, tag="pv", name=f"pv{h}_{b}_{p}",
                        padded_shape=[P, QB],
                    )
                pv = pv_of[u_prev * 2 + p]
                for c in chunks:
                    tt, jj, sl = pt_idx(p, c)
                    nc.tensor.matmul(
                        pv[:],
                        lhsT=vp[:, c, :],
                        rhs=pT[:, tt, jj, sl, :],
                        start=(c == 0),
                        stop=(c == NJ - 1),
                        skip_group_check=True,
                    )
                if close:
                    emit_output(h, b, p, pv, fast_tail=(u_prev == len(units) - 1))

            def emit_pv_steps(u_prev, s):
                p = s // 8
                emit_pv_chunks(
                    u_prev, p, (2 * (s % 8), 2 * (s % 8) + 1),
                    alloc=(s % 8 == 0), close=(s % 8 == 7),
                )

            def emit_exp(pT, sg, tt, jj):
                out_ap = pT[:, tt, jj, :, :]
                if (tt, jj) in dve_tiles:
                    nc.vector.tensor_scalar(
                        out_ap.bitcast(I16),
                        sg[:],
                        A16,
                        B16,
                        mybir.AluOpType.mult,
                        mybir.AluOpType.add,
                    )
                else:
                    nc.scalar.activation(
                        out_ap,
                        sg[:],
                        mybir.ActivationFunctionType.Exp,
                        scale=SCALE,
                    )

            for u, (h, b) in enumerate(units):
                if b == 0:
                    qTs = qTs_of[h]
                pT = ptpool.tile([P, 2, NG, 2, QB], BF16, tag="pT", name=f"pT{h}_{b}")
                pT_of[u] = pT
                for s in range(16):
                    jj, tt = s % 8, s // 8
                    kt_src = kTs if tt == 0 else kTs2
                    sg = sgpool.tile([P, 2, QB], F32, tag="sg", name=f"sg{h}_{b}_{s}")
                    nc.tensor.matmul(
                        sg[:, 0, :],
                        lhsT=kt_src[0:64, jj, :],
                        rhs=qTs[0:64, 4 * b : 4 * (b + 1), :],
                        start=True, stop=True,
                    )
                    nc.tensor.matmul(
                        sg[:, 1, :],
                        lhsT=kt_src[64:P, jj, :],
                        rhs=qTs[64:P, 4 * b : 4 * (b + 1), :],
                        start=True, stop=True,
                    )
                    if u == 0:
                        # fill the cold-start bubble: p0's even chunks are
                        # complete after step 7, thread them 1/step here.
                        if s >= 8:
                            emit_pv_chunks(0, 0, (2 * (s - 8),), alloc=(s == 8))
                    elif u == 1:
                        if s < 8:
                            emit_pv_chunks(0, 0, (2 * s + 1,), close=(s == 7))
                        else:
                            emit_pv_steps(0, s)
                    else:
                        emit_pv_steps(u - 1, s)
                    emit_exp(pT, sg, tt, jj)
                    if s == 3 and b == 1 and h + 1 < HPC:
                        qTs_of[h + 1] = load_qT(h + 1)

            for s in range(16):
                emit_pv_steps(len(units) - 1, s)


def _get_module(**cfg):
    key = tuple(sorted(cfg.items()))
    if key not in _CACHED:
        _CACHED[key] = _build_module(**cfg)
    return _CACHED[key]


def make_in_maps(Q, K, V):
    Q = np.asarray(Q, dtype=np.float32)
    K = np.asarray(K, dtype=np.float32)
    V = np.asarray(V, dtype=np.float32)
    in_maps = []
    for c in range(N_CORES):
        b = c // (N_CORES // B)
        h0 = HPC * (c % (N_CORES // B))
        in_maps.append(
            {
                "q": np.ascontiguousarray(Q[b, h0 : h0 + HPC]),
                "k": np.ascontiguousarray(K[b, 0]),
                "v": np.ascontiguousarray(V[b, 0]),
            }
        )
    return in_maps


def assemble_output(results):
    out = np.empty((B, H, S, D), dtype=np.float32)
    for c in range(N_CORES):
        b = c // (N_CORES // B)
        h0 = HPC * (c % (N_CORES // B))
        out[b, h0 : h0 + HPC] = results[c]["o"]
    return out


def kernel(Q, K, V):
    nc = _get_module(**DEFAULT_CFG)
    res = run_bass_kernel_spmd(nc, make_in_maps(Q, K, V), core_ids=list(range(N_CORES)))
    return assemble_output(res.results)

